# revision 1
# baseline (speedup 1.0000x reference)
"""Block-sparse linear kernel for Trainium2 (8 NeuronCores, Bass/Tile).

Computes out[n, ob*BS:(ob+1)*BS] += x[n, ib*BS:(ib+1)*BS] @ W[k]  for each
nonzero block k with indices (ob, ib), plus bias — data-parallel over the
flattened row dim N across 8 cores (weights/indices replicated).

Strategy (host-side schedule specialization from the index tensors):
  - Group input-blocks (ibs) into *families* with identical sets of
    output-blocks (obs).  Families whose obs-sets overlap are merged into
    *superfamilies* (zero-filled weight stacking keeps those correct).
  - Within a family, pair up ibs: a pair forms one K=128 stationary
    operand (the pair's two 64-feature slices of x, transposed host-side),
    streaming a [128, n_obs*64] stacked weight -> full PE utilization.
  - PSUM accumulates each superfamily-segment (<=16 obs = <=1024 f32 cols
    = 2 banks) over all its pairs/singles via matmul start/stop flags.
  - Output is laid out family-major (contiguous evictions); the host
    un-permutes output block columns and adds bias.
  - Matmuls run in float32r (TF32-like, ~1.5e-4 rel err, full PE rate).

The device kernel does: 2 input DMAs, matmul accumulation, PSUM->SBUF
evictions on ACT/DVE, 1 output DMA per 128-row tile.  All x transposition
and index logic happens on the host at schedule-build time.
"""

import numpy as np
from collections import defaultdict
from contextlib import ExitStack

from concourse import bass_utils, bacc, mybir
import concourse.tile as tile

N_CORES = 8
P = 128            # partitions / row-tile size
SEG_MAX_OBS = 16   # psum segment cap: 16 blocks * 64 = 1024 f32 = 2 banks
F32R = mybir.dt.float32r
F32 = mybir.dt.float32

# schedule-key -> (nc, meta) cache so repeated kernel() calls reuse the
# compiled module (and the NEFF cache underneath).
_CACHE = {}

# test harness introspection: last BassKernelResults
LAST_RESULT = None


def _build_schedule(N, F, OUT_F, BS, out_idx, in_idx):
    """Pure-index schedule: families, pairs, segments, layouts."""
    n_ib = F // BS
    n_ob = OUT_F // BS
    assert F % BS == 0 and OUT_F % BS == 0

    # (ob, ib) -> list of weight slots k (duplicates summed host-side)
    wslots = defaultdict(list)
    for k, (ob, ib) in enumerate(zip(out_idx, in_idx)):
        ob, ib = int(ob), int(ib)
        assert 0 <= ob < n_ob and 0 <= ib < n_ib
        wslots[(ob, ib)].append(k)

    obs_by_ib = defaultdict(set)
    for (ob, ib) in wslots:
        obs_by_ib[ib].add(ob)

    # families: ibs with identical obs sets
    fam_map = defaultdict(list)
    for ib in sorted(obs_by_ib):
        fam_map[frozenset(obs_by_ib[ib])].append(ib)
    families = [(sorted(obs), ibs) for obs, ibs in fam_map.items()]

    # union-find over obs to merge overlapping families into superfamilies
    parent = {}

    def find(a):
        while parent[a] != a:
            parent[a] = parent[parent[a]]
            a = parent[a]
        return a

    for obs, _ in families:
        for ob in obs:
            parent.setdefault(ob, ob)
        r = find(obs[0])
        for ob in obs[1:]:
            parent[find(ob)] = r
    sf_map = defaultdict(lambda: {"obs": set(), "fams": []})
    for obs, ibs in families:
        root = find(obs[0])
        sf_map[root]["obs"].update(obs)
        sf_map[root]["fams"].append((obs, ibs))
    superfams = sorted(sf_map.values(), key=lambda s: min(s["obs"]))

    # xt tile table: pairs (full K=128) and packed singles (K=64 halves)
    xt_tiles = []      # per tile: list of (rowbase, ib) entries
    unit_of = {}       # (fam_id, pair_idx) -> (tile_idx, rowbase, krows, ibs)
    singles = []       # deferred: (fam_key, ib)
    fam_units = defaultdict(list)   # fam key -> [(tile, rowbase, krows, ibs)]
    fam_id = 0
    fam_keys = {}
    for sf in superfams:
        for obs, ibs in sf["fams"]:
            key = fam_id
            fam_keys[key] = (tuple(obs), tuple(ibs))
            for i in range(0, len(ibs) - 1, 2):
                t = len(xt_tiles)
                xt_tiles.append([(0, ibs[i]), (64, ibs[i + 1])])
                fam_units[key].append((t, 0, 128, (ibs[i], ibs[i + 1])))
            if len(ibs) % 2:
                singles.append((key, ibs[-1]))
            fam_id += 1
    for j in range(0, len(singles), 2):
        t = len(xt_tiles)
        entries = [(0, singles[j][1])]
        fam_units[singles[j][0]].append((t, 0, 64, (singles[j][1],)))
        if j + 1 < len(singles):
            entries.append((64, singles[j + 1][1]))
            fam_units[singles[j + 1][0]].append((t, 64, 64, (singles[j + 1][1],)))
        xt_tiles.append(entries)

    # segments + ws layout + out layout
    # mm task: (psum_c0, psum_c1, tile, rowbase, krows, ws_c0, start, stop)
    segments = []   # per segment: dict(out_base, n_obs, obs, tasks)
    ws_blocks = []  # (ws_col, rowbase, ib_or_None, obs_list) for host fill
    ws_cols = 0
    out_cols = 0
    fid = 0
    for sf in superfams:
        sf_obs = sorted(sf["obs"])
        # family units of this superfamily, in deterministic order
        units = []
        base = fid
        for obs, ibs in sf["fams"]:
            units.append((fid, tuple(obs)))
            fid += 1
        for s0 in range(0, len(sf_obs), SEG_MAX_OBS):
            seg_obs = sf_obs[s0:s0 + SEG_MAX_OBS]
            L = len(seg_obs) * BS
            tasks = []
            all_units = []
            for key, fobs in units:
                for (t, rb, kr, uibs) in fam_units[key]:
                    all_units.append((t, rb, kr, uibs))
            for ui, (t, rb, kr, uibs) in enumerate(all_units):
                wc = ws_cols
                ws_blocks.append((wc, rb, uibs, seg_obs))
                for c0 in range(0, L, 512):
                    c1 = min(c0 + 512, L)
                    tasks.append((c0, c1, t, rb, kr, wc + c0,
                                  ui == 0, ui == len(all_units) - 1))
                ws_cols += L
            segments.append({"out_base": out_cols, "n_obs": len(seg_obs),
                             "obs": seg_obs, "tasks": tasks})
            out_cols += L

    n_pad = (-N) % (N_CORES * P)
    rows_per_core = (N + n_pad) // N_CORES
    rt_count = rows_per_core // P

    return {
        "N": N, "F": F, "OUT_F": OUT_F, "BS": BS,
        "wslots": dict(wslots),
        "xt_tiles": xt_tiles,
        "ws_blocks": ws_blocks, "ws_cols": ws_cols,
        "segments": segments, "out_cols": out_cols,
        "rows_per_core": rows_per_core, "rt_count": rt_count,
    }


def _build_nc(meta):
    """Emit the Bass/Tile module for a schedule (value-independent)."""
    Nc = meta["rows_per_core"]
    XTC = len(meta["xt_tiles"]) * Nc
    WSC = meta["ws_cols"]
    OUTC = meta["out_cols"]
    rt_count = meta["rt_count"]

    nc = bacc.Bacc("TRN2", target_bir_lowering=False, debug=False)
    xt_d = nc.dram_tensor("xt", [P, XTC], F32R, kind="ExternalInput")
    ws_d = nc.dram_tensor("ws", [P, WSC], F32R, kind="ExternalInput")
    out_d = nc.dram_tensor("out", [Nc, OUTC], F32, kind="ExternalOutput")

    with tile.TileContext(nc) as tc, ExitStack() as ctx:
        xt_pool = ctx.enter_context(tc.tile_pool(name="xt", bufs=1))
        ws_pool = ctx.enter_context(tc.tile_pool(name="ws", bufs=1))
        psum_pool = ctx.enter_context(tc.tile_pool(name="ps", bufs=4, space="PSUM"))
        out_pool = ctx.enter_context(tc.tile_pool(name="ot", bufs=2))

        xt = xt_pool.tile([P, XTC], F32R)
        ws = ws_pool.tile([P, WSC], F32R)
        nc.sync.dma_start(out=xt[:], in_=xt_d[:])
        nc.sync.dma_start(out=ws[:], in_=ws_d[:])

        ev = 0
        for rt in range(rt_count):
            out_sb = out_pool.tile([P, OUTC], F32)
            for seg in meta["segments"]:
                L = seg["n_obs"] * meta["BS"]
                psum = psum_pool.tile([P, 1024], F32)
                for (c0, c1, t, rb, kr, wc, start, stop) in seg["tasks"]:
                    lhsT = xt[rb:rb + kr, t * Nc + rt * P: t * Nc + (rt + 1) * P]
                    nc.tensor.matmul(
                        psum[:, c0:c1], lhsT, ws[rb:rb + kr, wc:wc + (c1 - c0)],
                        start=start, stop=stop)
                dst = out_sb[:, seg["out_base"]:seg["out_base"] + L]
                if ev % 2 == 0:
                    nc.scalar.copy(dst, psum[:, :L])
                else:
                    nc.vector.tensor_copy(out=dst, in_=psum[:, :L])
                ev += 1
            nc.sync.dma_start(out=out_d[rt * P:(rt + 1) * P, :], in_=out_sb[:])
    nc.compile()
    return nc


def _host_tensors(meta, x2, weight):
    """Build per-core xt and shared ws host arrays (values only)."""
    BS = meta["BS"]
    Nc = meta["rows_per_core"]
    Ntot = Nc * N_CORES

    if x2.shape[0] < Ntot:
        x2 = np.concatenate(
            [x2, np.zeros((Ntot - x2.shape[0], x2.shape[1]), np.float32)], axis=0)

    # ws (shared): [128, ws_cols]
    ws = np.zeros((P, meta["ws_cols"]), np.float32)
    wsum = {}
    for (ob_ib, ks) in meta["wslots"].items():
        w = weight[ks[0]]
        for k in ks[1:]:
            w = w + weight[k]
        wsum[ob_ib] = np.ascontiguousarray(w, dtype=np.float32)
    for (wc, rb, uibs, seg_obs) in meta["ws_blocks"]:
        for r, ib in enumerate(uibs):
            row0 = rb + r * 64
            for j, ob in enumerate(seg_obs):
                w = wsum.get((ob, ib))
                if w is not None:
                    ws[row0:row0 + 64, wc + j * BS: wc + (j + 1) * BS] = w

    # xt per core: [128, n_tiles*Nc]; tile t covers cols [t*Nc, (t+1)*Nc)
    xt_all = []
    for c in range(N_CORES):
        xs = x2[c * Nc:(c + 1) * Nc]           # [Nc, F]
        xt = np.zeros((P, len(meta["xt_tiles"]) * Nc), np.float32)
        for t, entries in enumerate(meta["xt_tiles"]):
            for (rbase, ib) in entries:
                xt[rbase:rbase + 64, t * Nc:(t + 1) * Nc] = \
                    xs[:, ib * BS:(ib + 1) * BS].T
        xt_all.append(np.ascontiguousarray(xt))
    return xt_all, np.ascontiguousarray(ws)


def kernel(**inputs):
    global LAST_RESULT
    x = np.asarray(inputs["x"], dtype=np.float32)
    weight = np.asarray(inputs["weight"], dtype=np.float32)
    bias = np.asarray(inputs["bias"], dtype=np.float32)
    out_idx = np.asarray(inputs["out_block_idx"]).astype(np.int64)
    in_idx = np.asarray(inputs["in_block_idx"]).astype(np.int64)

    B, S, F = x.shape
    N = B * S
    BS = weight.shape[1]
    OUT_F = bias.shape[0]
    x2 = np.ascontiguousarray(x.reshape(N, F))

    key = (N, F, OUT_F, BS, out_idx.tobytes(), in_idx.tobytes())
    if key not in _CACHE:
        meta = _build_schedule(N, F, OUT_F, BS, out_idx, in_idx)
        nc = _build_nc(meta)
        _CACHE[key] = (nc, meta)
    nc, meta = _CACHE[key]

    xt_all, ws = _host_tensors(meta, x2, weight)
    in_maps = [{"xt": xt_all[c], "ws": ws} for c in range(N_CORES)]
    res = bass_utils.run_bass_kernel_spmd(nc, in_maps, core_ids=list(range(N_CORES)))
    LAST_RESULT = res

    Nc = meta["rows_per_core"]
    dev = np.concatenate([res.results[c]["out"] for c in range(N_CORES)], axis=0)
    dev = dev[:N]  # drop row padding

    out = np.zeros((N, OUT_F), np.float32)
    for seg in meta["segments"]:
        b = seg["out_base"]
        for j, ob in enumerate(seg["obs"]):
            out[:, ob * BS:(ob + 1) * BS] = dev[:, b + j * BS: b + (j + 1) * BS]
    if bias.any():
        out += bias
    return out.reshape(B, S, OUT_F)


# revision 7
# speedup vs baseline: 1.1871x; 1.1871x over previous
"""Block-sparse linear kernel for Trainium2 (8 NeuronCores, Bass/Tile).

Computes out[n, ob*BS:(ob+1)*BS] += x[n, ib*BS:(ib+1)*BS] @ W[k]  for each
nonzero block k with indices (ob, ib), plus bias — data-parallel over the
flattened row dim N across 8 cores (weights/indices replicated).

Strategy (host-side schedule specialization from the index tensors):
  - Group input-blocks (ibs) into *families* with identical sets of
    output-blocks (obs).  Families whose obs-sets overlap are merged into
    *superfamilies* (zero-filled weight stacking keeps those correct).
  - Within a family, pair up ibs: a pair forms one K=128 stationary
    operand (the pair's two 64-feature slices of x, transposed host-side),
    streaming a [128, n_obs*64] stacked weight -> full PE utilization.
  - PSUM accumulates each superfamily-segment (<=16 obs = <=1024 f32 cols
    = 2 banks) over all its pairs/singles via matmul start/stop flags.
  - Output is laid out family-major (contiguous evictions); the host
    un-permutes output block columns and adds bias.
  - Matmuls run in float32r (TF32-like, ~1.5e-4 rel err, full PE rate).

The device kernel does: 2 input DMAs, matmul accumulation, PSUM->SBUF
evictions on ACT/DVE, 1 output DMA per 128-row tile.  All x transposition
and index logic happens on the host at schedule-build time.
"""

import numpy as np
from collections import defaultdict
from contextlib import ExitStack

from concourse import bass_utils, bacc, mybir
import concourse.tile as tile

N_CORES = 8
P = 128            # partitions / row-tile size
SEG_MAX_OBS = 16   # psum segment cap: 16 blocks * 64 = 1024 f32 = 2 banks
F32R = mybir.dt.float32r
F32 = mybir.dt.float32

# schedule-key -> (nc, meta) cache so repeated kernel() calls reuse the
# compiled module (and the NEFF cache underneath).
_CACHE = {}

# test harness introspection: last BassKernelResults
LAST_RESULT = None


def _build_schedule(N, F, OUT_F, BS, out_idx, in_idx):
    """Pure-index schedule: families, pairs, segments, layouts."""
    n_ib = F // BS
    n_ob = OUT_F // BS
    assert F % BS == 0 and OUT_F % BS == 0

    # (ob, ib) -> list of weight slots k (duplicates summed host-side)
    wslots = defaultdict(list)
    for k, (ob, ib) in enumerate(zip(out_idx, in_idx)):
        ob, ib = int(ob), int(ib)
        assert 0 <= ob < n_ob and 0 <= ib < n_ib
        wslots[(ob, ib)].append(k)

    obs_by_ib = defaultdict(set)
    for (ob, ib) in wslots:
        obs_by_ib[ib].add(ob)

    # families: ibs with identical obs sets
    fam_map = defaultdict(list)
    for ib in sorted(obs_by_ib):
        fam_map[frozenset(obs_by_ib[ib])].append(ib)
    families = [(sorted(obs), ibs) for obs, ibs in fam_map.items()]

    # union-find over obs to merge overlapping families into superfamilies
    parent = {}

    def find(a):
        while parent[a] != a:
            parent[a] = parent[parent[a]]
            a = parent[a]
        return a

    for obs, _ in families:
        for ob in obs:
            parent.setdefault(ob, ob)
        r = find(obs[0])
        for ob in obs[1:]:
            parent[find(ob)] = r
    sf_map = defaultdict(lambda: {"obs": set(), "fams": []})
    for obs, ibs in families:
        root = find(obs[0])
        sf_map[root]["obs"].update(obs)
        sf_map[root]["fams"].append((obs, ibs))
    superfams = sorted(sf_map.values(), key=lambda s: min(s["obs"]))

    # xt tile table: pairs (full K=128) and packed singles (K=64 halves)
    xt_tiles = []      # per tile: list of (rowbase, ib) entries
    unit_of = {}       # (fam_id, pair_idx) -> (tile_idx, rowbase, krows, ibs)
    singles = []       # deferred: (fam_key, ib)
    fam_units = defaultdict(list)   # fam key -> [(tile, rowbase, krows, ibs)]
    fam_id = 0
    fam_keys = {}
    for sf in superfams:
        for obs, ibs in sf["fams"]:
            key = fam_id
            fam_keys[key] = (tuple(obs), tuple(ibs))
            for i in range(0, len(ibs) - 1, 2):
                t = len(xt_tiles)
                xt_tiles.append([(0, ibs[i]), (64, ibs[i + 1])])
                fam_units[key].append((t, 0, 128, (ibs[i], ibs[i + 1])))
            if len(ibs) % 2:
                singles.append((key, ibs[-1]))
            fam_id += 1
    for j in range(0, len(singles), 2):
        t = len(xt_tiles)
        entries = [(0, singles[j][1])]
        fam_units[singles[j][0]].append((t, 0, 64, (singles[j][1],)))
        if j + 1 < len(singles):
            entries.append((64, singles[j + 1][1]))
            fam_units[singles[j + 1][0]].append((t, 64, 64, (singles[j + 1][1],)))
        xt_tiles.append(entries)

    # segments + ws layout + out layout
    # mm task: (psum_c0, psum_c1, tile, rowbase, krows, ws_c0, start, stop)
    segments = []   # per segment: dict(out_base, n_obs, obs, tasks)
    ws_blocks = []  # (ws_col, rowbase, ib_or_None, obs_list) for host fill
    ws_cols = 0
    out_cols = 0
    fid = 0
    for sf in superfams:
        sf_obs = sorted(sf["obs"])
        # family units of this superfamily, in deterministic order
        units = []
        base = fid
        for obs, ibs in sf["fams"]:
            units.append((fid, tuple(obs)))
            fid += 1
        for s0 in range(0, len(sf_obs), SEG_MAX_OBS):
            seg_obs = sf_obs[s0:s0 + SEG_MAX_OBS]
            L = len(seg_obs) * BS
            tasks = []
            all_units = []
            for key, fobs in units:
                for (t, rb, kr, uibs) in fam_units[key]:
                    all_units.append((t, rb, kr, uibs))
            seg_ws0 = ws_cols
            unit_ws = []
            for ui, (t, rb, kr, uibs) in enumerate(all_units):
                wc = ws_cols
                ws_blocks.append((wc, rb, uibs, seg_obs))
                unit_ws.append((wc, wc + L))
                for c0 in range(0, L, 512):
                    c1 = min(c0 + 512, L)
                    tasks.append((c0, c1, t, rb, kr, wc + c0,
                                  ui == 0, ui == len(all_units) - 1))
                ws_cols += L
            segments.append({"out_base": out_cols, "n_obs": len(seg_obs),
                             "obs": seg_obs, "tasks": tasks,
                             "ws0": seg_ws0, "ws1": ws_cols,
                             "unit_ws": unit_ws,
                             "tiles": sorted({tk[2] for tk in tasks})})
            out_cols += L

    n_pad = (-N) % (N_CORES * P)
    rows_per_core = (N + n_pad) // N_CORES
    rt_count = rows_per_core // P

    # input-DMA load plan in consumption order: ("ws"|"xt", c0, c1).
    # xt entries are tile-index ranges; first segment's ws goes per-unit so
    # the very first matmul only waits on a small chunk.
    load_plan = []
    seen_tiles = set()
    for si, seg in enumerate(segments):
        if si == 0:
            for (a, b) in seg["unit_ws"]:
                load_plan.append(("ws", a, b))
        else:
            load_plan.append(("ws", seg["ws0"], seg["ws1"]))
        new_t = [t for t in seg["tiles"] if t not in seen_tiles]
        seen_tiles.update(new_t)
        # merge consecutive tile indices into ranges
        i = 0
        while i < len(new_t):
            j = i
            while j + 1 < len(new_t) and new_t[j + 1] == new_t[j] + 1:
                j += 1
            load_plan.append(("xt", new_t[i], new_t[j] + 1))
            i = j + 1

    return {
        "N": N, "F": F, "OUT_F": OUT_F, "BS": BS,
        "wslots": dict(wslots),
        "xt_tiles": xt_tiles,
        "ws_blocks": ws_blocks, "ws_cols": ws_cols,
        "segments": segments, "out_cols": out_cols,
        "rows_per_core": rows_per_core, "rt_count": rt_count,
        "load_plan": load_plan,
    }


def _build_nc(meta):
    """Emit the Bass/Tile module for a schedule (value-independent)."""
    Nc = meta["rows_per_core"]
    XTC = len(meta["xt_tiles"]) * Nc
    WSC = meta["ws_cols"]
    OUTC = meta["out_cols"]
    rt_count = meta["rt_count"]

    nc = bacc.Bacc("TRN2", target_bir_lowering=False, debug=False)
    xt_d = nc.dram_tensor("xt", [P, XTC], F32R, kind="ExternalInput")
    ws_d = nc.dram_tensor("ws", [P, WSC], F32R, kind="ExternalInput")
    out_d = nc.dram_tensor("out", [Nc, OUTC], F32, kind="ExternalOutput")

    import os
    n_warm = int(os.environ.get("KWARM", "10"))

    with tile.TileContext(nc) as tc, ExitStack() as ctx:
        xt_pool = ctx.enter_context(tc.tile_pool(name="xt", bufs=1))
        ws_pool = ctx.enter_context(tc.tile_pool(name="ws", bufs=1))
        warm_pool = ctx.enter_context(tc.tile_pool(name="wm", bufs=1))
        psum_pool = ctx.enter_context(tc.tile_pool(name="ps", bufs=4, space="PSUM"))
        out_pool = ctx.enter_context(tc.tile_pool(name="ot", bufs=2))

        xt = xt_pool.tile([P, XTC], F32R)
        ws = ws_pool.tile([P, WSC], F32R)

        # PE warm-up: dummy matmuls with no DMA deps run during the input
        # load and flip HAM to 8/8 before the first real matmul.
        if n_warm:
            wsb = warm_pool.tile([P, 512], F32R)
            nc.vector.memset(wsb[:].bitcast(F32), 0)
            wps = psum_pool.tile([P, 1024], F32, tag="mm")
            for _ in range(n_warm):
                nc.tensor.matmul(wps[:, :512], wsb[:, :P], wsb[:, :512],
                                 start=True, stop=True)

        # chunked input DMAs in first-use order so matmuls start early
        for (kind, a, b) in meta["load_plan"]:
            if kind == "ws":
                nc.sync.dma_start(out=ws[:, a:b], in_=ws_d[:, a:b])
            else:
                nc.sync.dma_start(out=xt[:, a * Nc:b * Nc], in_=xt_d[:, a * Nc:b * Nc])

        ev = 0
        for rt in range(rt_count):
            out_sb = out_pool.tile([P, OUTC], F32)
            flushed = 0
            for si, seg in enumerate(meta["segments"]):
                L = seg["n_obs"] * meta["BS"]
                psum = psum_pool.tile([P, 1024], F32, tag="mm")
                for (c0, c1, t, rb, kr, wc, start, stop) in seg["tasks"]:
                    lhsT = xt[rb:rb + kr, t * Nc + rt * P: t * Nc + (rt + 1) * P]
                    nc.tensor.matmul(
                        psum[:, c0:c1], lhsT, ws[rb:rb + kr, wc:wc + (c1 - c0)],
                        start=start, stop=stop)
                dst = out_sb[:, seg["out_base"]:seg["out_base"] + L]
                if ev % 2 == 0:
                    nc.scalar.copy(dst, psum[:, :L])
                else:
                    nc.vector.tensor_copy(out=dst, in_=psum[:, :L])
                ev += 1
                # flush evicted output in ~0.5-1MB chunks to overlap the
                # store DMA with remaining compute
                done = seg["out_base"] + L
                if done - flushed >= 2048 or si == len(meta["segments"]) - 1:
                    nc.sync.dma_start(
                        out=out_d[rt * P:(rt + 1) * P, flushed:done],
                        in_=out_sb[:, flushed:done])
                    flushed = done
    nc.compile()
    return nc


def _host_tensors(meta, x2, weight):
    """Build per-core xt and shared ws host arrays (values only)."""
    BS = meta["BS"]
    Nc = meta["rows_per_core"]
    Ntot = Nc * N_CORES

    if x2.shape[0] < Ntot:
        x2 = np.concatenate(
            [x2, np.zeros((Ntot - x2.shape[0], x2.shape[1]), np.float32)], axis=0)

    # ws (shared): [128, ws_cols]
    ws = np.zeros((P, meta["ws_cols"]), np.float32)
    wsum = {}
    for (ob_ib, ks) in meta["wslots"].items():
        w = weight[ks[0]]
        for k in ks[1:]:
            w = w + weight[k]
        wsum[ob_ib] = np.ascontiguousarray(w, dtype=np.float32)
    for (wc, rb, uibs, seg_obs) in meta["ws_blocks"]:
        for r, ib in enumerate(uibs):
            row0 = rb + r * 64
            for j, ob in enumerate(seg_obs):
                w = wsum.get((ob, ib))
                if w is not None:
                    ws[row0:row0 + 64, wc + j * BS: wc + (j + 1) * BS] = w

    # xt per core: [128, n_tiles*Nc]; tile t covers cols [t*Nc, (t+1)*Nc)
    xt_all = []
    for c in range(N_CORES):
        xs = x2[c * Nc:(c + 1) * Nc]           # [Nc, F]
        xt = np.zeros((P, len(meta["xt_tiles"]) * Nc), np.float32)
        for t, entries in enumerate(meta["xt_tiles"]):
            for (rbase, ib) in entries:
                xt[rbase:rbase + 64, t * Nc:(t + 1) * Nc] = \
                    xs[:, ib * BS:(ib + 1) * BS].T
        xt_all.append(np.ascontiguousarray(xt))
    return xt_all, np.ascontiguousarray(ws)


def kernel(**inputs):
    global LAST_RESULT
    x = np.asarray(inputs["x"], dtype=np.float32)
    weight = np.asarray(inputs["weight"], dtype=np.float32)
    bias = np.asarray(inputs["bias"], dtype=np.float32)
    out_idx = np.asarray(inputs["out_block_idx"]).astype(np.int64)
    in_idx = np.asarray(inputs["in_block_idx"]).astype(np.int64)

    B, S, F = x.shape
    N = B * S
    BS = weight.shape[1]
    OUT_F = bias.shape[0]
    x2 = np.ascontiguousarray(x.reshape(N, F))

    key = (N, F, OUT_F, BS, out_idx.tobytes(), in_idx.tobytes())
    if key not in _CACHE:
        meta = _build_schedule(N, F, OUT_F, BS, out_idx, in_idx)
        nc = _build_nc(meta)
        _CACHE[key] = (nc, meta)
    nc, meta = _CACHE[key]

    xt_all, ws = _host_tensors(meta, x2, weight)
    in_maps = [{"xt": xt_all[c], "ws": ws} for c in range(N_CORES)]
    res = bass_utils.run_bass_kernel_spmd(nc, in_maps, core_ids=list(range(N_CORES)))
    LAST_RESULT = res

    Nc = meta["rows_per_core"]
    dev = np.concatenate([res.results[c]["out"] for c in range(N_CORES)], axis=0)
    dev = dev[:N]  # drop row padding

    out = np.zeros((N, OUT_F), np.float32)
    for seg in meta["segments"]:
        b = seg["out_base"]
        for j, ob in enumerate(seg["obs"]):
            out[:, ob * BS:(ob + 1) * BS] = dev[:, b + j * BS: b + (j + 1) * BS]
    if bias.any():
        out += bias
    return out.reshape(B, S, OUT_F)
